# revision 7
# baseline (speedup 1.0000x reference)
"""Trainium2 Bass kernel for nn_MultiHeadAttention_22883585753377 (v2).

Reference semantics (torch legacy): softmax over the HEADS axis (dim=1) of
the [B,H,S,S] score tensor, scale = sqrt(KEY_DIM)=32.

Sharding: 8 cores = (batch b, query-quarter r). Each core handles b = c//4,
512 query rows, all 16 heads. NEW in v2: the K and V projections are sharded
4-way across the cores of a batch group (each core projects only its own 512
keys) and exchanged with two AllGathers (replica groups [[0-3],[4-7]]) that
overlap the Q projection on the PE. Key order is permutation-invariant for
this attention (no mask), so every core simply consumes the gathered chunks
in rank order.

Engine budget (per core): ACT does only exp (the 16.7M-element softmax
numerator, ~1elem/cyc/lane floor) + some PSUM->SBUF copies; DVE does the
cross-head denominator tree, reciprocal (reciprocal_approx_fast) and 12/16
heads of the normalize-multiply; GpSimd does the remaining 4 heads; PE does
projections (bias via ones-outer-product rows), scores (row-tiled K=64 pairs
when SCORE_MODE=1) and AV (col-tiled M=64, as before).
"""

import numpy as np

B = 2
S = 1024 * 2
D = 1024
H = 16
DH = 64
SQ = 512  # query rows per core
QH = 256  # q processed per half
KC = 128  # k-chunk (partition dim of scores^T tiles)
NKC = S // KC  # 16
KC4 = 512  # projection / shard chunk
NKC4 = S // KC4  # 4
SCALE = 1.0 / 32.0  # 1/sqrt(KEY_DIM)
LAG1 = 1
LAG2 = 3
SCORE_MODE = 0  # 0: one K=128 zero-padded MM per pair; 1: two row-tiled K=64 MMs
MSPLIT = 16  # heads multiplied on DVE; rest on GpSimd
RECIP_MODE = 0  # 0: ACT Ln + Exp(-x); 1: DVE reciprocal_approx_fast

_CACHE = {}


def _legalize_waits(nc):
    """This container's walrus encodes at most ONE semaphore wait per
    instruction; Tile emits up to ~10. Split the excess onto same-engine nops
    inserted immediately before the instruction."""
    import bass_rust

    ctr = [0]
    for bb in nc.main_func.blocks:
        insts = list(bb.instructions)
        out = []
        changed = False
        for ins in insts:
            si = ins.sync_info
            waits = list(si.on_wait) if si is not None and si.on_wait else []
            if len(waits) > 1:
                changed = True
                upd = list(si.on_update) if si.on_update else []
                for w in waits[:-1]:
                    ctr[0] += 1
                    nop = bass_rust.InstNoOp(
                        name=f"I-wsplit-{ctr[0]}", ins=[], outs=[]
                    )
                    nop.engine = ins.engine
                    nop.bass_nofuse = True
                    nop.sync_info = bass_rust.SyncInfo(on_wait=[w], on_update=[])
                    out.append(nop)
                ins.sync_info = bass_rust.SyncInfo(
                    on_wait=[waits[-1]], on_update=upd
                )
            out.append(ins)
        if changed:
            bb.instructions = out


def _bcast_cols(ap, rep, seg):
    """View a [128, seg] AP as [128, rep, seg] with stride 0 on the middle
    (repeat) dim."""
    import dataclasses

    a = [tuple(x) for x in ap.ap]
    assert a[-1][1] == seg, a
    return dataclasses.replace(ap, ap=[a[0], (0, rep), a[-1]])


def _split_cols(ap, rep, seg):
    """View a [128, rep*seg] contiguous AP as [128, rep, seg]."""
    import dataclasses

    a = [tuple(x) for x in ap.ap]
    assert a[-1] == (1, rep * seg), a
    return dataclasses.replace(ap, ap=[a[0], (seg, rep), (1, seg)])


def _ins_dim(ap, stride, rep):
    """Insert a middle (stride, rep) dim into a 2D AP."""
    import dataclasses

    a = [tuple(x) for x in ap.ap]
    assert len(a) == 2, a
    return dataclasses.replace(ap, ap=[a[0], (stride, rep), a[-1]])


def _build(legalize=True, score_mode=SCORE_MODE, msplit=MSPLIT, recip_mode=RECIP_MODE):
    import concourse.bass as bass
    import concourse.mybir as mybir
    import concourse.tile as tile

    bf16 = mybir.dt.bfloat16
    f32 = mybir.dt.float32
    AF = mybir.ActivationFunctionType

    nc = bass.Bass()

    # --- I/O (per core: its q shard and its KEY shard, both transposed) ----
    qT_d = nc.dram_tensor("qT", [D, SQ], bf16, kind="ExternalInput")
    kTs_d = nc.dram_tensor("kTs", [D, KC4], bf16, kind="ExternalInput")
    vTs_d = nc.dram_tensor("vTs", [D, KC4], bf16, kind="ExternalInput")
    wq_d = nc.dram_tensor("wq", [D, D], bf16, kind="ExternalInput")
    wk_d = nc.dram_tensor("wk", [D, D], bf16, kind="ExternalInput")
    wv_d = nc.dram_tensor("wv", [D, D], bf16, kind="ExternalInput")
    wo_d = nc.dram_tensor("wo", [D, D], bf16, kind="ExternalInput")
    bq_d = nc.dram_tensor("bq", [1, D], bf16, kind="ExternalInput")
    bk_d = nc.dram_tensor("bk", [1, D], bf16, kind="ExternalInput")
    bv_d = nc.dram_tensor("bv", [1, D], bf16, kind="ExternalInput")
    bo_d = nc.dram_tensor("bo", [1, D], bf16, kind="ExternalInput")
    out_d = nc.dram_tensor("out", [SQ, D], f32, kind="ExternalOutput")

    RG = [[0, 1, 2, 3], [4, 5, 6, 7]]

    with tile.TileContext(nc) as tc:
        with (
            tc.tile_pool(name="persist", bufs=1) as persist,
            tc.tile_pool(name="consts", bufs=1) as consts,
            tc.tile_pool(name="ccdram", bufs=1, space="DRAM") as ccdram,
        ):
            # K^T per (chunk, pair): [128 feat, 512 k]; pair p = heads 2p,2p+1
            KT = [
                [persist.tile([128, KC4], bf16, tag=f"KT{c}_{p}", name=f"KT{c}_{p}")
                 for p in range(8)]
                for c in range(NKC4)
            ]
            # V natural [S,D] as 16 x [128, D]
            V = [persist.tile([128, D], bf16, tag=f"V{s}", name=f"V{s}") for s in range(16)]
            # Q^T per head, head h's 64 features at rows (h%2)*64; other rows
            # are don't-care in score_mode=1 (row-tiled K=64 contraction) but
            # must be ZERO in score_mode=0 (K=128 pair matmul).
            QTb = persist.tile([128, H * SQ], bf16, tag="QTb", name="QTb")
            # O^T per (qh, pair): [128 feat, 256 q]
            OT = [
                [persist.tile([128, QH], bf16, tag=f"OT{qh}_{p}", name=f"OT{qh}_{p}")
                 for p in range(8)]
                for qh in range(2)
            ]

            ones = consts.tile([1, KC4], bf16)
            nc.vector.memset(ones[:], 1.0)
            bq_s = consts.tile([1, D], bf16, tag="bq")
            bk_s = consts.tile([1, D], bf16, tag="bk")
            bv_s = consts.tile([1, D], bf16, tag="bv")
            bo_s = consts.tile([1, D], bf16, tag="bo")
            nc.sync.dma_start(bq_s[:], bq_d[:])
            nc.sync.dma_start(bk_s[:], bk_d[:])
            nc.sync.dma_start(bv_s[:], bv_d[:])
            nc.sync.dma_start(bo_s[:], bo_d[:])

            # Collective bounce buffers (HBM). AllGather concatenates the
            # four 128-row rank contributions along the first axis.
            ccink = ccdram.tile([128, 8 * KC4], bf16, name="ccink")
            ccoutk = ccdram.tile([512, 8 * KC4], bf16, name="ccoutk")
            ccinv = ccdram.tile([128, 4 * D], bf16, name="ccinv")
            ccoutv = ccdram.tile([512, 4 * D], bf16, name="ccoutv")

            # ---------------- Phase A: projections + gather ----------------
            with tc.tile_pool(name="wkv", bufs=1) as wkv:
                wkall = wkv.tile([128, 8 * D], bf16, tag="wkall", name="wkall")
                wvall = wkv.tile([128, 8 * D], bf16, tag="wvall", name="wvall")
                wqall = wkv.tile([128, 8 * D], bf16, tag="wqall", name="wqall")
                kraw = wkv.tile([128, 8 * KC4], bf16, tag="kraw", name="kraw")
                vraw = wkv.tile([128, 8 * KC4], bf16, tag="vraw", name="vraw")
                qraw = wkv.tile([128, 8 * SQ], bf16, tag="qraw", name="qraw")
                kstg = wkv.tile([128, 8 * KC4], bf16, tag="kstg", name="kstg")
                vstg = wkv.tile([128, 4 * D], bf16, tag="vstg", name="vstg")

                # DMA order: SP queue: wk, kraw, wv, vraw; ACT queue: wq, qraw.
                for d in range(8):
                    nc.sync.dma_start(wkall[:, d * D : (d + 1) * D], wk_d[d * 128 : (d + 1) * 128, :])
                for d in range(8):
                    nc.sync.dma_start(kraw[:, d * KC4 : (d + 1) * KC4], kTs_d[d * 128 : (d + 1) * 128, :])
                for d in range(8):
                    nc.scalar.dma_start(wqall[:, d * D : (d + 1) * D], wq_d[d * 128 : (d + 1) * 128, :])
                    nc.scalar.dma_start(qraw[:, d * SQ : (d + 1) * SQ], qT_d[d * 128 : (d + 1) * 128, :])
                for d in range(8):
                    nc.sync.dma_start(wvall[:, d * D : (d + 1) * D], wv_d[d * 128 : (d + 1) * 128, :])
                for d in range(8):
                    nc.sync.dma_start(vraw[:, d * KC4 : (d + 1) * KC4], vTs_d[d * 128 : (d + 1) * 128, :])

                with tc.tile_pool(name="proj_ps", bufs=2, space="PSUM") as projp:
                    # K projection of the local 512-key shard.
                    for f in range(8):
                        ps = projp.tile([128, KC4], f32, tag="pj")
                        for d in range(8):
                            nc.tensor.matmul(
                                ps[:],
                                wkall[:, d * D + f * 128 : d * D + (f + 1) * 128],
                                kraw[:, d * KC4 : (d + 1) * KC4],
                                start=(d == 0),
                                stop=False,
                            )
                        # bias: bk[f-chunk] as out-partition values x ones row
                        nc.tensor.matmul(
                            ps[:],
                            bk_s[0:1, f * 128 : (f + 1) * 128],
                            ones[0:1, :],
                            start=False,
                            stop=True,
                        )
                        nc.scalar.copy(kstg[:, f * KC4 : (f + 1) * KC4], ps[:])
                    nc.sync.dma_start(ccink[:], kstg[:])
                    nc.gpsimd.collective_compute(
                        "AllGather",
                        mybir.AluOpType.bypass,
                        replica_groups=RG,
                        ins=[ccink.opt()],
                        outs=[ccoutk.opt()],
                    )

                    # V projection of the local shard: 4 key-blocks x [128, D]
                    for sv in range(4):
                        for f2 in range(2):
                            pv = projp.tile([128, KC4], f32, tag="pj")
                            for d in range(8):
                                nc.tensor.matmul(
                                    pv[:],
                                    vraw[:, d * KC4 + sv * 128 : d * KC4 + (sv + 1) * 128],
                                    wvall[:, d * D + f2 * 512 : d * D + (f2 + 1) * 512],
                                    start=(d == 0),
                                    stop=False,
                                )
                            nc.tensor.matmul(
                                pv[:],
                                ones[0:1, 0:128],
                                bv_s[0:1, f2 * 512 : (f2 + 1) * 512],
                                start=False,
                                stop=True,
                            )
                            nc.vector.tensor_copy(
                                vstg[:, sv * D + f2 * 512 : sv * D + (f2 + 1) * 512],
                                pv[:],
                            )
                    nc.sync.dma_start(ccinv[:], vstg[:])
                    nc.gpsimd.collective_compute(
                        "AllGather",
                        mybir.AluOpType.bypass,
                        replica_groups=RG,
                        ins=[ccinv.opt()],
                        outs=[ccoutv.opt()],
                    )

                    # Q projection -> QTb (bias via PE, copy via ACT).
                    for f in range(8):
                        ps = projp.tile([128, SQ], f32, tag="pjq")
                        for d in range(8):
                            nc.tensor.matmul(
                                ps[:],
                                wqall[:, d * D + f * 128 : d * D + (f + 1) * 128],
                                qraw[:, d * SQ : (d + 1) * SQ],
                                start=(d == 0),
                                stop=False,
                            )
                        nc.tensor.matmul(
                            ps[:],
                            bq_s[0:1, f * 128 : (f + 1) * 128],
                            ones[0:1, :],
                            start=False,
                            stop=True,
                        )
                        nc.scalar.copy(
                            QTb[0:64, (2 * f) * SQ : (2 * f + 1) * SQ], ps[0:64, :]
                        )
                        nc.scalar.copy(
                            QTb[64:128, (2 * f + 1) * SQ : (2 * f + 2) * SQ],
                            ps[64:128, :],
                        )
                    if score_mode == 0:
                        for h in range(16):
                            r = (h % 2) * 64
                            nc.vector.memset(
                                QTb[64 - r : 128 - r, h * SQ : (h + 1) * SQ], 0.0
                            )

                # Read back the gathered projections (own chunk included —
                # key order is irrelevant, simplest uniform program).
                for g in range(4):
                    for p in range(8):
                        nc.sync.dma_start(
                            KT[g][p][:],
                            ccoutk[g * 128 : (g + 1) * 128, p * KC4 : (p + 1) * KC4],
                        )
                for g in range(4):
                    for j in range(4):
                        nc.sync.dma_start(
                            V[4 * g + j][:],
                            ccoutv[g * 128 : (g + 1) * 128, j * D : (j + 1) * D],
                        )

            # ---------------- fused attention helpers ----------------
            def attn_scores(qh, kc, escp, eexpp):
                """Scores + exp for one (qh, kc): 4 groups of 4 heads."""
                c, rr = kc // 4, kc % 4
                e = eexpp.tile([128, H * QH], bf16, tag="e")
                for g in range(4):
                    sc = escp.tile([128, 4 * QH], f32, tag="sc")
                    for pp in range(2):
                        p = 2 * g + pp
                        if score_mode == 1:
                            for hh in range(2):
                                nc.tensor.matmul(
                                    sc[:, (2 * pp + hh) * QH : (2 * pp + hh + 1) * QH],
                                    KT[c][p][hh * 64 : (hh + 1) * 64, rr * 128 : (rr + 1) * 128],
                                    QTb[hh * 64 : (hh + 1) * 64,
                                        (2 * p + hh) * SQ + qh * QH : (2 * p + hh) * SQ + qh * QH + QH],
                                    start=True,
                                    stop=True,
                                )
                        else:
                            mov = _ins_dim(
                                QTb[:, 2 * p * SQ + qh * QH : 2 * p * SQ + qh * QH + QH],
                                SQ, 2,
                            )
                            nc.tensor.matmul(
                                sc[:, pp * 2 * QH : (pp + 1) * 2 * QH],
                                KT[c][p][:, rr * 128 : (rr + 1) * 128],
                                mov,
                                start=True,
                                stop=True,
                            )
                    nc.scalar.activation(
                        e[:, g * 4 * QH : (g + 1) * 4 * QH],
                        sc[:],
                        AF.Exp,
                        scale=SCALE,
                    )
                return e

            def attn_tree(e, emid):
                """Cross-head denominator tree on DVE; final level -> f32."""
                t1 = emid.tile([128, 8 * QH], bf16, tag="t1", bufs=1)
                nc.vector.tensor_add(t1[:], e[:, : 8 * QH], e[:, 8 * QH :])
                t2 = emid.tile([128, 4 * QH], bf16, tag="t2", bufs=1)
                nc.vector.tensor_add(t2[:], t1[:, : 4 * QH], t1[:, 4 * QH :])
                t3 = emid.tile([128, 2 * QH], bf16, tag="t3", bufs=1)
                nc.vector.tensor_add(t3[:], t2[:, : 2 * QH], t2[:, 2 * QH :])
                den = emid.tile([128, QH], f32, tag="den")
                nc.vector.tensor_add(den[:], t3[:, :QH], t3[:, QH:])
                return den

            def attn_finish(e, den, emid, ewtsp):
                """Reciprocal + normalize-multiply split DVE/GpSimd.
                Emitted one iteration late."""
                r16 = emid.tile([128, QH], bf16, tag="r16")
                if recip_mode == 1:
                    rf = emid.tile([128, QH], f32, tag="rf", bufs=1)
                    nc.vector.reciprocal_approx_fast(out=rf[:], in_=den[:])
                    nc.vector.tensor_copy(r16[:], rf[:])
                else:
                    lden = emid.tile([128, QH], f32, tag="lden", bufs=1)
                    nc.scalar.activation(lden[:], den[:], AF.Ln)
                    nc.scalar.activation(r16[:], lden[:], AF.Exp, scale=-1.0)
                w = ewtsp.tile([128, H * QH], bf16, tag="w")
                nd = msplit
                nc.vector.tensor_mul(
                    _split_cols(w[:, : nd * QH], nd, QH),
                    _split_cols(e[:, : nd * QH], nd, QH),
                    _bcast_cols(r16[:], nd, QH),
                )
                if nd < H:
                    ng = H - nd
                    nc.gpsimd.tensor_mul(
                        _split_cols(w[:, nd * QH :], ng, QH),
                        _split_cols(e[:, nd * QH :], ng, QH),
                        _bcast_cols(r16[:], ng, QH),
                    )
                return w

            def attn_av(oacc, kc, w):
                for j in range(8):
                    cs = slice((j // 4) * QH, (j // 4 + 1) * QH)
                    for hh in range(2):
                        h = 2 * j + hh
                        nc.tensor.matmul(
                            oacc[j % 4][hh * 64 : (hh + 1) * 64, cs],
                            V[kc][:, h * 64 : (h + 1) * 64],
                            w[:, h * QH : (h + 1) * QH],
                            start=(kc == 0 and j < 4),
                            stop=(kc == NKC - 1),
                            skip_group_check=True,
                        )

            def oacc_flush(qh, oacc):
                for j in range(8):
                    cs = slice((j // 4) * QH, (j // 4 + 1) * QH)
                    nc.scalar.copy(OT[qh][j][:], oacc[j % 4][:, cs])

            # ---------------- Stage 1: attention qh=0 ----------------
            with (
                tc.tile_pool(name="sc_ps", bufs=2, space="PSUM") as scp,
                tc.tile_pool(name="oacc_ps", bufs=1, space="PSUM") as oaccp,
                tc.tile_pool(name="exp_sb", bufs=2) as expp,
                tc.tile_pool(name="wts_sb", bufs=1 + LAG1) as wtsp,
                tc.tile_pool(name="mid_sb", bufs=2) as mid,
            ):
                oaccA = [
                    oaccp.tile([128, 2 * QH], f32, tag=f"oA{i}", name=f"oA{i}")
                    for i in range(4)
                ]
                ering = [None] * NKC
                dring = [None] * NKC
                wring = [None] * NKC
                for kc in range(NKC):
                    e = attn_scores(0, kc, scp, expp)
                    ering[kc] = e
                    dring[kc] = attn_tree(e, mid)
                    if kc >= 1:
                        wring[kc - 1] = attn_finish(
                            ering[kc - 1], dring[kc - 1], mid, wtsp
                        )
                    if kc >= 1 + LAG1:
                        attn_av(oaccA, kc - 1 - LAG1, wring[kc - 1 - LAG1])
                wring[NKC - 1] = attn_finish(
                    ering[NKC - 1], dring[NKC - 1], mid, wtsp
                )
                for kc in range(NKC - 1 - LAG1, NKC):
                    attn_av(oaccA, kc, wring[kc])
                oacc_flush(0, oaccA)

            # ---------------- Stage 2: attention qh=1 + O proj ----------------
            with tc.tile_pool(name="wot_sb", bufs=1) as wot:
                woall = wot.tile([128, 8 * D], bf16, tag="woall", name="woall")
                for j in range(8):
                    nc.sync.dma_start(woall[:, j * D : (j + 1) * D], wo_d[j * 128 : (j + 1) * 128, :])

                with (
                    tc.tile_pool(name="sc2_ps", bufs=2, space="PSUM") as scp2,
                    tc.tile_pool(name="oacc2_ps", bufs=1, space="PSUM") as oaccp2,
                    tc.tile_pool(name="exp2_sb", bufs=4) as expp2,
                    tc.tile_pool(name="wts2_sb", bufs=1 + LAG2) as wtsp2,
                    tc.tile_pool(name="mid2_sb", bufs=3) as mid2,
                ):
                    oaccB = [
                        oaccp2.tile([128, 2 * QH], f32, tag=f"oB{i}", name=f"oB{i}")
                        for i in range(4)
                    ]
                    ering2 = [None] * NKC
                    dring2 = [None] * NKC
                    wring2 = [None] * NKC
                    for kc in range(NKC):
                        e = attn_scores(1, kc, scp2, expp2)
                        ering2[kc] = e
                        dring2[kc] = attn_tree(e, mid2)
                        if kc >= 1:
                            wring2[kc - 1] = attn_finish(
                                ering2[kc - 1], dring2[kc - 1], mid2, wtsp2
                            )
                        if kc >= 1 + LAG2:
                            attn_av(oaccB, kc - 1 - LAG2, wring2[kc - 1 - LAG2])
                    wring2[NKC - 1] = attn_finish(
                        ering2[NKC - 1], dring2[NKC - 1], mid2, wtsp2
                    )
                    for kc in range(NKC - 1 - LAG2, NKC):
                        attn_av(oaccB, kc, wring2[kc])
                    oacc_flush(1, oaccB)

                # Stage 3: output projection.
                with (
                    tc.tile_pool(name="pO1", bufs=2, space="PSUM") as pO1,
                    tc.tile_pool(name="osb", bufs=2) as osb,
                ):
                    for q4 in range(4):
                        qh, qr = q4 // 2, q4 % 2
                        po = pO1.tile([128, D], f32, tag="po")
                        for j in range(8):
                            for f2 in range(2):
                                nc.tensor.matmul(
                                    po[:, f2 * 512 : (f2 + 1) * 512],
                                    OT[qh][j][:, qr * 128 : (qr + 1) * 128],
                                    woall[:, j * D + f2 * 512 : j * D + (f2 + 1) * 512],
                                    start=(j == 0),
                                    stop=False,
                                )
                        for f2 in range(2):
                            nc.tensor.matmul(
                                po[:, f2 * 512 : (f2 + 1) * 512],
                                ones[0:1, 0:128],
                                bo_s[0:1, f2 * 512 : (f2 + 1) * 512],
                                start=False,
                                stop=True,
                            )
                        ob = osb.tile([128, D], f32, tag="ob")
                        nc.vector.tensor_copy(ob[:], po[:])
                        nc.sync.dma_start(out_d[q4 * 128 : (q4 + 1) * 128, :], ob[:])

    if legalize:
        _legalize_waits(nc)
    return nc


def _prep_inputs(inputs):
    import ml_dtypes

    bf16 = ml_dtypes.bfloat16
    q = np.asarray(inputs["queries"], np.float32)
    k = np.asarray(inputs["keys"], np.float32)
    v = np.asarray(inputs["values"], np.float32)
    Wq = np.asarray(inputs["Wq"], np.float32).astype(bf16)
    Wk = np.asarray(inputs["Wk"], np.float32).astype(bf16)
    Wv = np.asarray(inputs["Wv"], np.float32).astype(bf16)
    Wo = np.asarray(inputs["Wo"], np.float32).astype(bf16)
    bq = np.asarray(inputs["bq"], np.float32).astype(bf16).reshape(1, D)
    bk = np.asarray(inputs["bk"], np.float32).astype(bf16).reshape(1, D)
    bv = np.asarray(inputs["bv"], np.float32).astype(bf16).reshape(1, D)
    bo = np.asarray(inputs["bo"], np.float32).astype(bf16).reshape(1, D)

    in_maps = []
    for c in range(8):
        b, r = c // 4, c % 4
        qT = np.ascontiguousarray(q[b, r * SQ : (r + 1) * SQ, :].T).astype(bf16)
        kTs = np.ascontiguousarray(k[b, r * KC4 : (r + 1) * KC4, :].T).astype(bf16)
        vTs = np.ascontiguousarray(v[b, r * KC4 : (r + 1) * KC4, :].T).astype(bf16)
        in_maps.append(
            {
                "qT": qT,
                "kTs": kTs,
                "vTs": vTs,
                "wq": Wq,
                "wk": Wk,
                "wv": Wv,
                "wo": Wo,
                "bq": bq,
                "bk": bk,
                "bv": bv,
                "bo": bo,
            }
        )
    return in_maps


def run(inputs, trace=False, trace_kwargs=None):
    """Build (cached), run on 8 cores, return (output, BassKernelResults)."""
    from concourse.bass_utils import run_bass_kernel_spmd

    if "nc" not in _CACHE:
        _CACHE["nc"] = _build()
    nc = _CACHE["nc"]
    in_maps = _prep_inputs(inputs)
    res = run_bass_kernel_spmd(
        nc,
        in_maps,
        core_ids=list(range(8)),
        trace=trace,
        **(trace_kwargs or {}),
    )
    out = np.empty((B, S, D), np.float32)
    for c in range(8):
        b, r = c // 4, c % 4
        out[b, r * SQ : (r + 1) * SQ, :] = res.results[c]["out"]
    return out, res


def kernel(**inputs) -> np.ndarray:
    out, _ = run(inputs, trace=False)
    return out
